# revision 1
# baseline (speedup 1.0000x reference)
import sys

if "/opt/trn_rl_repo" not in sys.path:
    sys.path.insert(0, "/opt/trn_rl_repo")

import numpy as np

B, S, D, NH, DH = 4, 2048, 768, 12, 64
NHL = 6        # heads per core
NPAIR = 3      # head pairs per core
NCH = 6        # d_model chunks of 128
NT = 16        # seq tiles of 128
NSQ = 4        # sq chunks of 512

_CACHE = {}


def build_nc(body_reps=1):
    import concourse.tile as tile
    from concourse import mybir, bacc

    f32 = mybir.dt.float32
    bf16 = mybir.dt.bfloat16
    AF = mybir.ActivationFunctionType

    nc = bacc.Bacc("TRN2", target_bir_lowering=False, debug=False)

    xT_d = nc.dram_tensor("xT", [128, NCH * S], bf16, kind="ExternalInput")
    wq_d = nc.dram_tensor("wq", [128, NPAIR * NCH * 128], bf16, kind="ExternalInput")
    wk_d = nc.dram_tensor("wk", [128, NPAIR * NCH * 128], bf16, kind="ExternalInput")
    wv_d = nc.dram_tensor("wv", [128, NCH * 384], bf16, kind="ExternalInput")
    wo_d = nc.dram_tensor("wo", [128, NPAIR * 768], bf16, kind="ExternalInput")
    bqk_d = nc.dram_tensor("bqk", [128, 6], f32, kind="ExternalInput")
    bvb_d = nc.dram_tensor("bvb", [128, 384], f32, kind="ExternalInput")
    mask_d = nc.dram_tensor("maskT", [128, 128], bf16, kind="ExternalInput")
    ones_d = nc.dram_tensor("onesr", [1, 64], bf16, kind="ExternalInput")
    out_d = nc.dram_tensor("out", [S, D], f32, kind="ExternalOutput")

    with tile.TileContext(nc) as tc:
        for rep in range(body_reps):
            _emit_body(nc, tc, tile, mybir, rep,
                       xT_d, wq_d, wk_d, wv_d, wo_d, bqk_d, bvb_d, mask_d, ones_d, out_d)

    nc.compile()
    return nc


def _emit_body(nc, tc, tile, mybir, rep,
               xT_d, wq_d, wk_d, wv_d, wo_d, bqk_d, bvb_d, mask_d, ones_d, out_d):
    f32 = mybir.dt.float32
    bf16 = mybir.dt.bfloat16
    AF = mybir.ActivationFunctionType
    R = f"r{rep}"

    with (
        tc.tile_pool(name=f"sb{rep}", bufs=1) as sb,
        tc.tile_pool(name=f"psum{rep}", bufs=1, space="PSUM") as psum,
    ):
        # ---- constants ----
        wo_sb = sb.tile([128, NPAIR, 768], bf16, tag="wo")
        bqk_sb = sb.tile([128, 6], f32, tag="bqk")
        bvb_sb = sb.tile([128, 384], f32, tag="bvb")
        mask_sb = sb.tile([128, 128], bf16, tag="mask")
        ones_sb = sb.tile([1, 64], bf16, tag="ones")
        xT_sb = sb.tile([128, NCH, S], bf16, tag="xT")
        wq_sb = sb.tile([128, NPAIR, NCH, 128], bf16, tag="wq")
        wk_sb = sb.tile([128, NPAIR, NCH, 128], bf16, tag="wk")
        wv_sb = sb.tile([128, NCH, 384], bf16, tag="wv")

        for c in range(NCH):
            for hf in range(2):
                nc.sync.dma_start(
                    xT_sb[:, c, hf * (S // 2):(hf + 1) * (S // 2)],
                    xT_d[:, c * S + hf * (S // 2):c * S + (hf + 1) * (S // 2)],
                )
        nc.sync.dma_start(wv_sb[:], wv_d[:].rearrange("k (c f) -> k c f", c=NCH))
        nc.sync.dma_start(bvb_sb[:], bvb_d[:])
        nc.sync.dma_start(wq_sb[:], wq_d[:].rearrange("k (p c m) -> k p c m", p=NPAIR, c=NCH))
        nc.sync.dma_start(wk_sb[:], wk_d[:].rearrange("k (p c m) -> k p c m", p=NPAIR, c=NCH))
        nc.sync.dma_start(bqk_sb[:], bqk_d[:])
        nc.sync.dma_start(mask_sb[:], mask_d[:])
        nc.sync.dma_start(ones_sb[:], ones_d[:])
        nc.sync.dma_start(wo_sb[:], wo_d[:].rearrange("k (p d) -> k p d", p=NPAIR))

        qT_sb = [sb.tile([128, S], bf16, tag=f"qT{p}", name=f"qT{p}{R}") for p in range(NPAIR)]
        kT_sb = [sb.tile([128, S], bf16, tag=f"kT{p}", name=f"kT{p}{R}") for p in range(NPAIR)]
        v_sb = [sb.tile([128, NHL, 65], bf16, tag=f"v{j}", name=f"v{j}{R}") for j in range(NT)]
        zT_sb = [sb.tile([128, S], bf16, tag=f"zT{p}", name=f"zT{p}{R}") for p in range(NPAIR)]

        # ---- v projection (natural layout), bias add + ones col ----
        for j in range(NT):
            pv = psum.tile([128, 512], f32, name=f"pv{j}{R}", tag="pj", bufs=2)
            for c in range(NCH):
                nc.tensor.matmul(
                    pv[:, 0:384],
                    lhsT=xT_sb[:, c, j * 128:(j + 1) * 128],
                    rhs=wv_sb[:, c, :],
                    start=(c == 0),
                    stop=(c == NCH - 1),
                )
            nc.vector.memset(v_sb[j][:, :, 64:65], 1.0)
            nc.vector.tensor_add(
                v_sb[j][:, :, 0:64],
                pv[:, 0:384].rearrange("k (h e) -> k h e", h=NHL),
                bvb_sb[:].rearrange("k (h e) -> k h e", h=NHL),
            )

        def qk_proj(p):
            for half, (w_sb, dst) in enumerate(((wq_sb, qT_sb[p]), (wk_sb, kT_sb[p]))):
                for nq in range(NSQ):
                    ps = psum.tile([128, 512], f32, name=f"pr{p}_{half}_{nq}{R}", tag="pj", bufs=2)
                    for c in range(NCH):
                        nc.tensor.matmul(
                            ps[:],
                            lhsT=w_sb[:, p, c, :],
                            rhs=xT_sb[:, c, nq * 512:(nq + 1) * 512],
                            start=(c == 0),
                            stop=(c == NCH - 1),
                        )
                    nc.vector.tensor_scalar_add(
                        dst[:, nq * 512:(nq + 1) * 512],
                        ps[:],
                        bqk_sb[:, 2 * p + half:2 * p + half + 1],
                    )

        def attention(p, cq):
            if True:
                jmax = 4 * cq + 3
                pz = [
                    psum.tile([65, 512], f32, name=f"pz{p}_{cq}_{h}{R}", tag=f"pz{h}", bufs=1)
                    for h in range(2)
                ]
                # j=0 first (writes the full bank with start=True), then the
                # diagonal blocks (whose mask dep chain is longest), then the rest
                jorder = [0] + list(range(max(4 * cq, 1), jmax + 1)) + list(range(1, 4 * cq))
                pts = {}
                for j in jorder:
                    sqs = max(512 * cq, 128 * j)
                    n = 512 - (sqs - 512 * cq)
                    # one [128, 1024] psum tile: cols 0:512 head A, 512:1024 head B
                    ps = psum.tile([128, 2, 512], f32, name=f"st{p}_{cq}_{j}{R}", tag="st", bufs=2)
                    for h in range(2):
                        nc.tensor.matmul(
                            ps[:, h, :n],
                            lhsT=kT_sb[p][64 * h:64 * h + 64, j * 128:(j + 1) * 128],
                            rhs=qT_sb[p][64 * h:64 * h + 64, sqs:sqs + n],
                        )
                    pt = sb.tile([128, 2, 512], bf16, name=f"pt{p}_{cq}_{j}{R}", tag="pt", bufs=16)
                    pts[j] = pt
                    nc.scalar.activation(pt[:, :, :n], ps[:, :, :n], AF.Exp, scale=0.125)
                    if j >= 4 * cq:
                        # diagonal block: causal mask (keep sk <= sq)
                        nc.gpsimd.tensor_mul(pt[:, 0, 0:128], pt[:, 0, 0:128], mask_sb[:])
                        nc.gpsimd.tensor_mul(pt[:, 1, 0:128], pt[:, 1, 0:128], mask_sb[:])
                for ji, j in enumerate(jorder):
                    sqs = max(512 * cq, 128 * j)
                    off = sqs - 512 * cq
                    n = 512 - off
                    for h in range(2):
                        nc.tensor.matmul(
                            pz[h][:, off:off + n],
                            lhsT=v_sb[j][:, p * 2 + h, :],
                            rhs=pts[j][:, h, :n],
                            start=(ji == 0),
                            stop=(ji == jmax),
                        )
                for h in range(2):
                    # evacuate pz with one copy (frees the PSUM bank fast)
                    zh = sb.tile([65, 512], f32, name=f"zh{p}_{cq}_{h}{R}", tag="zh", bufs=2)
                    nc.vector.tensor_copy(zh[:], pz[h][:])
                    rec = sb.tile([1, 512], bf16, name=f"rec{p}_{cq}_{h}{R}", tag="rec", bufs=2)
                    with nc.allow_low_precision(reason="softmax denom recip feeds bf16 bcast matmul"):
                        nc.vector.reciprocal(rec[:], zh[64:65, :])
                    pb = psum.tile([64, 512], f32, name=f"pb{p}_{cq}_{h}{R}", tag="pj", bufs=2)
                    nc.tensor.matmul(
                        pb[:],
                        lhsT=ones_sb[:],
                        rhs=rec[:],
                    )
                    nc.vector.tensor_mul(
                        zT_sb[p][64 * h:64 * h + 64, 512 * cq:512 * (cq + 1)],
                        pb[:],
                        zh[0:64, :],
                    )

        def outproj(t):
            osb = sb.tile([128, 768], f32, name=f"osb{t}{R}", tag="osb", bufs=4)
            for dh in range(2):
                po = psum.tile([128, 384], f32, name=f"po{t}_{dh}{R}", tag="pj", bufs=2)
                for p in range(NPAIR):
                    nc.tensor.matmul(
                        po[:],
                        lhsT=zT_sb[p][:, t * 128:(t + 1) * 128],
                        rhs=wo_sb[:, p, dh * 384:(dh + 1) * 384],
                        start=(p == 0),
                        stop=(p == NPAIR - 1),
                    )
                nc.vector.tensor_copy(osb[:, dh * 384:(dh + 1) * 384], po[:])
            nc.sync.dma_start(out_d[t * 128:(t + 1) * 128, :], osb[:])

        # interleave: qk projections ahead; cq-outer so out-projection of
        # chunk cq overlaps attention of chunk cq+1
        qk_proj(0)
        qk_proj(1)
        for cq in range(NSQ):
            attention(0, cq)
            if cq == 0:
                qk_proj(2)
            attention(1, cq)
            attention(2, cq)
            for t in range(4 * cq, 4 * cq + 4):
                outproj(t)


def make_in_maps(normalized_resid_pre, W_Q, b_Q, W_K, b_K, W_V, b_V, W_O, b_O):
    import ml_dtypes

    bf = ml_dtypes.bfloat16
    x = np.asarray(normalized_resid_pre, dtype=np.float32)
    W_Q = np.asarray(W_Q, np.float32)
    W_K = np.asarray(W_K, np.float32)
    W_V = np.asarray(W_V, np.float32)
    W_O = np.asarray(W_O, np.float32)
    b_Q = np.asarray(b_Q, np.float32)
    b_K = np.asarray(b_K, np.float32)
    b_V = np.asarray(b_V, np.float32)

    mask = (np.arange(128)[:, None] <= np.arange(128)[None, :]).astype(bf)

    xT_by_batch = []
    for b in range(B):
        xT = np.ascontiguousarray(x[b].T)            # [768, 2048]
        xT = xT.reshape(NCH, 128, S).transpose(1, 0, 2).reshape(128, NCH * S)
        xT_by_batch.append(np.ascontiguousarray(xT.astype(bf)))

    def wqk_arrange(W, h0):
        # [128(k), NPAIR, NCH, 128(m = hp*64+e)]
        w = W[h0:h0 + NHL].reshape(NPAIR, 2, NCH, 128, DH)   # p, hp, c, k, e
        w = w.transpose(0, 2, 3, 1, 4)                        # p, c, k, hp, e
        return np.ascontiguousarray(
            w.reshape(NPAIR, NCH, 128, 128).transpose(2, 0, 1, 3).reshape(128, NPAIR * NCH * 128).astype(bf))

    def wv_arrange(W, h0):
        # [128(k), NCH, 384(f = h*64+e)]
        w = W[h0:h0 + NHL].reshape(NHL, NCH, 128, DH)         # h, c, k, e
        w = w.transpose(2, 1, 0, 3)                           # k, c, h, e
        return np.ascontiguousarray(w.reshape(128, NCH * 384).astype(bf))

    def wo_arrange(W, h0):
        # [128(k = hp*64+e), NPAIR*768]
        w = W[h0:h0 + NHL].reshape(NPAIR, 2, DH, D)           # p, hp, e, d
        w = w.transpose(1, 2, 0, 3)                           # hp, e, p, d
        return np.ascontiguousarray(w.reshape(128, NPAIR * D).astype(bf))

    in_maps = []
    for core in range(8):
        b = core // 2
        h0 = (core % 2) * NHL
        bqk = np.zeros((128, 6), np.float32)
        for p in range(NPAIR):
            bqk[:, 2 * p] = b_Q[h0 + 2 * p:h0 + 2 * p + 2].reshape(128)
            bqk[:, 2 * p + 1] = b_K[h0 + 2 * p:h0 + 2 * p + 2].reshape(128)
        bvb = np.broadcast_to(b_V[h0:h0 + NHL].reshape(1, 384), (128, 384))
        in_maps.append({
            "xT": xT_by_batch[b],
            "wq": wqk_arrange(W_Q, h0),
            "wk": wqk_arrange(W_K, h0),
            "wv": wv_arrange(W_V, h0),
            "wo": wo_arrange(W_O, h0),
            "bqk": bqk,
            "bvb": np.ascontiguousarray(bvb),
            "maskT": mask,
            "onesr": np.ones((1, 64), bf),
        })
    return in_maps


def gather(results, b_O):
    out = np.zeros((B, S, D), np.float32)
    for b in range(B):
        out[b] = results[2 * b]["out"] + results[2 * b + 1]["out"]
    out += np.asarray(b_O, np.float32)[None, None, :]
    return out


def kernel(normalized_resid_pre, W_Q, b_Q, W_K, b_K, W_V, b_V, W_O, b_O, _trace=False):
    from concourse.bass_utils import run_bass_kernel_spmd

    if "nc" not in _CACHE:
        _CACHE["nc"] = build_nc()
    nc = _CACHE["nc"]
    in_maps = make_in_maps(normalized_resid_pre, W_Q, b_Q, W_K, b_K, W_V, b_V, W_O, b_O)
    res = run_bass_kernel_spmd(nc, in_maps, list(range(8)), trace=_trace)
    _CACHE["last_result"] = res
    return gather(res.results, b_O)



# revision 2
# speedup vs baseline: 6.9766x; 6.9766x over previous
import sys

if "/opt/trn_rl_repo" not in sys.path:
    sys.path.insert(0, "/opt/trn_rl_repo")

import numpy as np

B, S, D, NH, DH = 4, 2048, 768, 12, 64
NHL = 6        # heads per core
NPAIR = 3      # head pairs per core
NCH = 6        # d_model chunks of 128
NT = 16        # seq tiles of 128
NSQ = 4        # sq chunks of 512

_CACHE = {}


def build_nc(body_reps=1):
    import concourse.tile as tile
    from concourse import mybir, bacc

    f32 = mybir.dt.float32
    bf16 = mybir.dt.bfloat16
    AF = mybir.ActivationFunctionType

    nc = bacc.Bacc("TRN2", target_bir_lowering=False, debug=False)

    xT_d = nc.dram_tensor("xT", [128, NCH * S], bf16, kind="ExternalInput")
    wq_d = nc.dram_tensor("wq", [128, NPAIR * NCH * 128], bf16, kind="ExternalInput")
    wk_d = nc.dram_tensor("wk", [128, NPAIR * NCH * 128], bf16, kind="ExternalInput")
    wv_d = nc.dram_tensor("wv", [128, NCH * 384], bf16, kind="ExternalInput")
    wo_d = nc.dram_tensor("wo", [128, NPAIR * 768], bf16, kind="ExternalInput")
    bqk_d = nc.dram_tensor("bqk", [128, 6], f32, kind="ExternalInput")
    bvb_d = nc.dram_tensor("bvb", [128, 384], f32, kind="ExternalInput")
    mask_d = nc.dram_tensor("maskT", [128, 128], bf16, kind="ExternalInput")
    ones_d = nc.dram_tensor("onesr", [1, 64], bf16, kind="ExternalInput")
    out_d = nc.dram_tensor("out", [S, D], f32, kind="ExternalOutput")

    with tile.TileContext(nc) as tc:
        for rep in range(body_reps):
            _emit_body(nc, tc, tile, mybir, rep,
                       xT_d, wq_d, wk_d, wv_d, wo_d, bqk_d, bvb_d, mask_d, ones_d, out_d)

    nc.compile()
    return nc


def _emit_body(nc, tc, tile, mybir, rep,
               xT_d, wq_d, wk_d, wv_d, wo_d, bqk_d, bvb_d, mask_d, ones_d, out_d):
    f32 = mybir.dt.float32
    bf16 = mybir.dt.bfloat16
    AF = mybir.ActivationFunctionType
    R = f"r{rep}"

    with (
        tc.tile_pool(name=f"sb{rep}", bufs=1) as sb,
        tc.tile_pool(name=f"psum{rep}", bufs=1, space="PSUM") as psum,
    ):
        # ---- constants ----
        wo_sb = sb.tile([128, NPAIR, 768], bf16, tag="wo")
        bqk_sb = sb.tile([128, 6], f32, tag="bqk")
        bvb_sb = sb.tile([128, 384], f32, tag="bvb")
        mask_sb = sb.tile([128, 128], bf16, tag="mask")
        ones_sb = sb.tile([1, 64], bf16, tag="ones")
        xT_sb = sb.tile([128, NCH, S], bf16, tag="xT")
        wq_sb = sb.tile([128, NPAIR, NCH, 128], bf16, tag="wq")
        wk_sb = sb.tile([128, NPAIR, NCH, 128], bf16, tag="wk")
        wv_sb = sb.tile([128, NCH, 384], bf16, tag="wv")

        # xT streams on the SP queue (hf-outer so the first 1024 seq positions
        # of every d_model chunk land first); weights go down the Activation
        # hwdge queue concurrently so v/qk projections can start early.
        for hf in range(2):
            for c in range(NCH):
                nc.sync.dma_start(
                    xT_sb[:, c, hf * (S // 2):(hf + 1) * (S // 2)],
                    xT_d[:, c * S + hf * (S // 2):c * S + (hf + 1) * (S // 2)],
                )
        nc.scalar.dma_start(wv_sb[:], wv_d[:].rearrange("k (c f) -> k c f", c=NCH))
        nc.scalar.dma_start(bvb_sb[:], bvb_d[:])
        nc.scalar.dma_start(wq_sb[:], wq_d[:].rearrange("k (p c m) -> k p c m", p=NPAIR, c=NCH))
        nc.scalar.dma_start(wk_sb[:], wk_d[:].rearrange("k (p c m) -> k p c m", p=NPAIR, c=NCH))
        nc.scalar.dma_start(bqk_sb[:], bqk_d[:])
        nc.scalar.dma_start(mask_sb[:], mask_d[:])
        nc.scalar.dma_start(ones_sb[:], ones_d[:])
        nc.scalar.dma_start(wo_sb[:], wo_d[:].rearrange("k (p d) -> k p d", p=NPAIR))

        qT_sb = [sb.tile([128, S], bf16, tag=f"qT{p}", name=f"qT{p}{R}") for p in range(NPAIR)]
        kT_sb = [sb.tile([128, S], bf16, tag=f"kT{p}", name=f"kT{p}{R}") for p in range(NPAIR)]
        v_sb = [sb.tile([128, NHL, 65], bf16, tag=f"v{j}", name=f"v{j}{R}") for j in range(NT)]
        zT_sb = [sb.tile([128, S], bf16, tag=f"zT{p}", name=f"zT{p}{R}") for p in range(NPAIR)]

        # ---- v projection (natural layout), bias add + ones col ----
        for j in range(NT):
            pv = psum.tile([128, 512], f32, name=f"pv{j}{R}", tag="pj", bufs=2)
            for c in range(NCH):
                nc.tensor.matmul(
                    pv[:, 0:384],
                    lhsT=xT_sb[:, c, j * 128:(j + 1) * 128],
                    rhs=wv_sb[:, c, :],
                    start=(c == 0),
                    stop=(c == NCH - 1),
                )
            nc.vector.memset(v_sb[j][:, :, 64:65], 1.0)
            nc.vector.tensor_add(
                v_sb[j][:, :, 0:64],
                pv[:, 0:384].rearrange("k (h e) -> k h e", h=NHL),
                bvb_sb[:].rearrange("k (h e) -> k h e", h=NHL),
            )

        def qk_proj(p):
            for half, (w_sb, dst) in enumerate(((wq_sb, qT_sb[p]), (wk_sb, kT_sb[p]))):
                for nq in range(NSQ):
                    ps = psum.tile([128, 512], f32, name=f"pr{p}_{half}_{nq}{R}", tag="pj", bufs=2)
                    for c in range(NCH):
                        nc.tensor.matmul(
                            ps[:],
                            lhsT=w_sb[:, p, c, :],
                            rhs=xT_sb[:, c, nq * 512:(nq + 1) * 512],
                            start=(c == 0),
                            stop=(c == NCH - 1),
                        )
                    nc.vector.tensor_scalar_add(
                        dst[:, nq * 512:(nq + 1) * 512],
                        ps[:],
                        bqk_sb[:, 2 * p + half:2 * p + half + 1],
                    )

        def attention(p, cq):
            if True:
                jmax = 4 * cq + 3
                pz = [
                    psum.tile([65, 512], f32, name=f"pz{p}_{cq}_{h}{R}", tag=f"pz{h}", bufs=1)
                    for h in range(2)
                ]
                # j=0 first (writes the full bank with start=True), then the
                # diagonal blocks (whose mask dep chain is longest), then the rest
                jorder = [0] + list(range(max(4 * cq, 1), jmax + 1)) + list(range(1, 4 * cq))
                pts = {}
                for j in jorder:
                    sqs = max(512 * cq, 128 * j)
                    n = 512 - (sqs - 512 * cq)
                    # one [128, 1024] psum tile: cols 0:512 head A, 512:1024 head B
                    ps = psum.tile([128, 2, 512], f32, name=f"st{p}_{cq}_{j}{R}", tag="st", bufs=2)
                    for h in range(2):
                        nc.tensor.matmul(
                            ps[:, h, :n],
                            lhsT=kT_sb[p][64 * h:64 * h + 64, j * 128:(j + 1) * 128],
                            rhs=qT_sb[p][64 * h:64 * h + 64, sqs:sqs + n],
                        )
                    pt = sb.tile([128, 2, 512], bf16, name=f"pt{p}_{cq}_{j}{R}", tag="pt", bufs=16)
                    pts[j] = pt
                    nc.scalar.activation(pt[:, :, :n], ps[:, :, :n], AF.Exp, scale=0.125)
                    if j >= 4 * cq:
                        # diagonal block: causal mask (keep sk <= sq)
                        nc.gpsimd.tensor_mul(pt[:, 0, 0:128], pt[:, 0, 0:128], mask_sb[:])
                        nc.gpsimd.tensor_mul(pt[:, 1, 0:128], pt[:, 1, 0:128], mask_sb[:])
                for ji, j in enumerate(jorder):
                    sqs = max(512 * cq, 128 * j)
                    off = sqs - 512 * cq
                    n = 512 - off
                    for h in range(2):
                        nc.tensor.matmul(
                            pz[h][:, off:off + n],
                            lhsT=v_sb[j][:, p * 2 + h, :],
                            rhs=pts[j][:, h, :n],
                            start=(ji == 0),
                            stop=(ji == jmax),
                        )
                for h in range(2):
                    # normalize straight out of PSUM: reciprocal of the ones-row
                    # denominator, broadcast it across partitions on GpSimd,
                    # multiply against the unnormalized z rows in place.
                    rec = sb.tile([1, 512], bf16, name=f"rec{p}_{cq}_{h}{R}", tag="rec", bufs=2)
                    with nc.allow_low_precision(reason="softmax denom recip feeds bf16 bcast"):
                        nc.vector.reciprocal(rec[:], pz[h][64:65, :])
                    recb = sb.tile([64, 512], bf16, name=f"recb{p}_{cq}_{h}{R}", tag="recb", bufs=2)
                    nc.gpsimd.partition_broadcast(recb[:], rec[:])
                    nc.vector.tensor_mul(
                        zT_sb[p][64 * h:64 * h + 64, 512 * cq:512 * (cq + 1)],
                        recb[:],
                        pz[h][0:64, :],
                    )

        def outproj(t):
            osb = sb.tile([128, 768], f32, name=f"osb{t}{R}", tag="osb", bufs=4)
            for dh in range(2):
                po = psum.tile([128, 384], f32, name=f"po{t}_{dh}{R}", tag="pj", bufs=2)
                for p in range(NPAIR):
                    nc.tensor.matmul(
                        po[:],
                        lhsT=zT_sb[p][:, t * 128:(t + 1) * 128],
                        rhs=wo_sb[:, p, dh * 384:(dh + 1) * 384],
                        start=(p == 0),
                        stop=(p == NPAIR - 1),
                    )
                nc.vector.tensor_copy(osb[:, dh * 384:(dh + 1) * 384], po[:])
            nc.sync.dma_start(out_d[t * 128:(t + 1) * 128, :], osb[:])

        # interleave: qk projections ahead; cq-outer so out-projection of
        # chunk cq overlaps attention of chunk cq+1
        qk_proj(0)
        qk_proj(1)
        for cq in range(NSQ):
            attention(0, cq)
            if cq == 0:
                qk_proj(2)
            attention(1, cq)
            attention(2, cq)
            for t in range(4 * cq, 4 * cq + 4):
                outproj(t)


def make_in_maps(normalized_resid_pre, W_Q, b_Q, W_K, b_K, W_V, b_V, W_O, b_O):
    import ml_dtypes

    bf = ml_dtypes.bfloat16
    x = np.asarray(normalized_resid_pre, dtype=np.float32)
    W_Q = np.asarray(W_Q, np.float32)
    W_K = np.asarray(W_K, np.float32)
    W_V = np.asarray(W_V, np.float32)
    W_O = np.asarray(W_O, np.float32)
    b_Q = np.asarray(b_Q, np.float32)
    b_K = np.asarray(b_K, np.float32)
    b_V = np.asarray(b_V, np.float32)

    mask = (np.arange(128)[:, None] <= np.arange(128)[None, :]).astype(bf)

    xT_by_batch = []
    for b in range(B):
        xT = np.ascontiguousarray(x[b].T)            # [768, 2048]
        xT = xT.reshape(NCH, 128, S).transpose(1, 0, 2).reshape(128, NCH * S)
        xT_by_batch.append(np.ascontiguousarray(xT.astype(bf)))

    def wqk_arrange(W, h0):
        # [128(k), NPAIR, NCH, 128(m = hp*64+e)]
        w = W[h0:h0 + NHL].reshape(NPAIR, 2, NCH, 128, DH)   # p, hp, c, k, e
        w = w.transpose(0, 2, 3, 1, 4)                        # p, c, k, hp, e
        return np.ascontiguousarray(
            w.reshape(NPAIR, NCH, 128, 128).transpose(2, 0, 1, 3).reshape(128, NPAIR * NCH * 128).astype(bf))

    def wv_arrange(W, h0):
        # [128(k), NCH, 384(f = h*64+e)]
        w = W[h0:h0 + NHL].reshape(NHL, NCH, 128, DH)         # h, c, k, e
        w = w.transpose(2, 1, 0, 3)                           # k, c, h, e
        return np.ascontiguousarray(w.reshape(128, NCH * 384).astype(bf))

    def wo_arrange(W, h0):
        # [128(k = hp*64+e), NPAIR*768]
        w = W[h0:h0 + NHL].reshape(NPAIR, 2, DH, D)           # p, hp, e, d
        w = w.transpose(1, 2, 0, 3)                           # hp, e, p, d
        return np.ascontiguousarray(w.reshape(128, NPAIR * D).astype(bf))

    in_maps = []
    for core in range(8):
        b = core // 2
        h0 = (core % 2) * NHL
        bqk = np.zeros((128, 6), np.float32)
        for p in range(NPAIR):
            bqk[:, 2 * p] = b_Q[h0 + 2 * p:h0 + 2 * p + 2].reshape(128)
            bqk[:, 2 * p + 1] = b_K[h0 + 2 * p:h0 + 2 * p + 2].reshape(128)
        bvb = np.broadcast_to(b_V[h0:h0 + NHL].reshape(1, 384), (128, 384))
        in_maps.append({
            "xT": xT_by_batch[b],
            "wq": wqk_arrange(W_Q, h0),
            "wk": wqk_arrange(W_K, h0),
            "wv": wv_arrange(W_V, h0),
            "wo": wo_arrange(W_O, h0),
            "bqk": bqk,
            "bvb": np.ascontiguousarray(bvb),
            "maskT": mask,
            "onesr": np.ones((1, 64), bf),
        })
    return in_maps


def gather(results, b_O):
    out = np.zeros((B, S, D), np.float32)
    for b in range(B):
        out[b] = results[2 * b]["out"] + results[2 * b + 1]["out"]
    out += np.asarray(b_O, np.float32)[None, None, :]
    return out


def kernel(normalized_resid_pre, W_Q, b_Q, W_K, b_K, W_V, b_V, W_O, b_O, _trace=False):
    from concourse.bass_utils import run_bass_kernel_spmd

    if "nc" not in _CACHE:
        _CACHE["nc"] = build_nc()
    nc = _CACHE["nc"]
    in_maps = make_in_maps(normalized_resid_pre, W_Q, b_Q, W_K, b_K, W_V, b_V, W_O, b_O)
    res = run_bass_kernel_spmd(nc, in_maps, list(range(8)), trace=_trace)
    _CACHE["last_result"] = res
    return gather(res.results, b_O)

